# revision 2
# baseline (speedup 1.0000x reference)
"""Trainium2 Bass kernel v2 for nn_GATNet (GraphDTA-style GAT network).

Changes vs v1 baseline:
- dma_gather (1 instr/block) replaces per-chunk indirect DMAs
- f16 xtab (256B rows), 640-wide f16 shared table (h2p + asrc2/adst2 aux)
- ELU computed as y=elu(x)+1 with 2 DVE + 1 ACT ops via ones-row matmul
  bias trick; +1 offsets folded into downstream weights on host
- adtab gather replaced by aux-dst dma_gather from the shared table
- branch-interleaved emission: AG(1) hidden behind branch-2 compute,
  AG(2) behind L2-att(1) + cell MLP
"""
import numpy as np

import concourse.bacc as bacc
import concourse.tile as tile
import concourse.mybir as mybir
from concourse.masks import make_identity

F32 = mybir.dt.float32
F16 = mybir.dt.float16
I16 = mybir.dt.int16
Alu = mybir.AluOpType
Act = mybir.ActivationFunctionType

N_CORES = 8
N_NODES = 6000
N_EDGES = 36000
N_GRAPHS = 128
NV = 750
NBLK = 6
GPC = 16
H1, C1 = 10, 1024
D0 = 78
XTW = 128         # xtab row: 0:78 x | 78 one | 79:89 asrc | pad   (f16, 256B)
T2 = 640          # table2 row: 0:512 h2p | 512:514 asrc2 f32 | 514:516 adst2 f32 | pad
NEG_SLOPE = 0.2


# ---------------------------------------------------------------- host prep
def prep_edges(ei, alpha=None):
    """Returns n_ch, src_idx, dst_loc, dst_glo, alpha_pe.
    alpha: [42000, H1] per-edge normalized attention (sorted-edge order) or
    None."""
    src = np.concatenate([ei[0], np.arange(N_NODES, dtype=ei.dtype)])
    dst = np.concatenate([ei[1], np.arange(N_NODES, dtype=ei.dtype)])
    order = np.argsort(dst, kind="stable")
    src, dst = src[order], dst[order]
    cores = []
    for c in range(N_CORES):
        m = (dst >= NV * c) & (dst < NV * (c + 1))
        s, d = src[m], dst[m] - NV * c
        a = alpha[m] if alpha is not None else None
        blocks = []
        for b in range(NBLK):
            mb = (d >= 128 * b) & (d < min(128 * (b + 1), NV))
            blocks.append((s[mb], d[mb], a[mb] if a is not None else None))
        cores.append(blocks)
    n_ch = [max(1, int(np.ceil(max(len(cores[c][b][0]) for c in range(N_CORES))
                               / 128))) for b in range(NBLK)]
    nch_tot = sum(n_ch)
    src_idx = np.zeros((N_CORES, 128, nch_tot), np.int64)
    dst_loc = np.full((N_CORES, 128, nch_tot), -1.0, np.float32)
    dst_glo = np.full((N_CORES, 128, nch_tot), NV, np.int64)
    alpha_pe = np.zeros((N_CORES, 128, nch_tot, H1), np.float16)
    for c in range(N_CORES):
        off = 0
        for b in range(NBLK):
            s, d, a = cores[c][b]
            npad = n_ch[b] * 128
            sp = np.zeros(npad, np.int64)
            dp = np.full(npad, -1.0, np.float64)
            gp = np.full(npad, NV, np.int64)
            sp[:len(s)] = s
            dp[:len(d)] = d - 128 * b
            gp[:len(d)] = d
            src_idx[c, :, off:off + n_ch[b]] = sp.reshape(n_ch[b], 128).T
            dst_loc[c, :, off:off + n_ch[b]] = dp.reshape(n_ch[b], 128).T
            dst_glo[c, :, off:off + n_ch[b]] = gp.reshape(n_ch[b], 128).T
            if a is not None:
                ap_ = np.zeros((npad, H1), np.float16)
                ap_[:len(s)] = a.astype(np.float16)
                alpha_pe[c, :, off:off + n_ch[b], :] = (
                    ap_.reshape(n_ch[b], 128, H1).transpose(1, 0, 2))
            off += n_ch[b]
    return n_ch, src_idx, dst_loc, dst_glo, alpha_pe


def wrap_idx(flat):
    """flat [n] (n%16==0) -> [128, n//16] int16 in dma_gather layout:
    logical index i lives at partition i%16, col i//16 (replicated 8x)."""
    n = len(flat)
    w = flat.reshape(n // 16, 16).T.astype(np.int16)
    return np.tile(w, (8, 1)).copy()


def host_prep(inputs):
    inp = {k: np.asarray(v) for k, v in inputs.items()}
    Hh = {}
    meta = {}
    f16 = np.float16
    W1 = inp["W1"].astype(np.float32)
    W13 = W1.reshape(D0, H1, C1)
    A_s = np.einsum("dhc,hc->dh", W13, inp["a_src1"].astype(np.float32))
    A_d = np.einsum("dhc,hc->dh", W13, inp["a_dst1"].astype(np.float32))
    for k in (1, 2):
        x = inp[f"x{k}"].astype(np.float32)
        asrc = x @ A_s
        adst = x @ A_d
        xtab = np.zeros((N_NODES, XTW), f16)
        xtab[:, 0:D0] = x.astype(f16)
        xtab[:, D0] = 1.0
        xtab[:, 79:89] = asrc.astype(f16)
        Hh[f"xtab{k}"] = xtab
        # host-side L1 attention: alpha = softmax_dst(lrelu(asrc[s]+adst[d]))
        ei = inp[f"edge_index{k}"]
        s_all = np.concatenate([ei[0], np.arange(N_NODES, dtype=ei.dtype)])
        d_all = np.concatenate([ei[1], np.arange(N_NODES, dtype=ei.dtype)])
        order = np.argsort(d_all, kind="stable")
        s_s, d_s = s_all[order], d_all[order]
        e_ed = asrc[s_s] + adst[d_s]                       # [E+N, H1]
        e_ed = np.where(e_ed > 0, e_ed, NEG_SLOPE * e_ed)
        w_ed = np.exp(e_ed)
        dsum = np.zeros((N_NODES, H1), np.float64)
        np.add.at(dsum, d_s, w_ed)
        alpha = (w_ed / np.maximum(dsum[d_s], 1e-30)).astype(np.float32)
        n_ch, src_idx, dst_loc, dst_glo, alpha_pe = prep_edges(ei, alpha)
        meta[f"n_ch{k}"] = n_ch
        nch_tot = sum(n_ch)
        L = nch_tot * 8
        l1ix = np.zeros((N_CORES, 128, L), np.int16)
        l2dx = np.zeros((N_CORES, 128, L), np.int16)
        for c in range(N_CORES):
            sflat = src_idx[c].T.reshape(-1)          # i = ch*128 + p
            l1ix[c] = wrap_idx(sflat)
            dglo = dst_glo[c].T.reshape(-1)
            dglo = np.where(dglo == NV, 0, dglo + NV * c)
            l2dx[c] = wrap_idx(dglo)
        Hh[f"l1ix{k}"] = l1ix
        Hh[f"l2dx{k}"] = l2dx
        Hh[f"alphat{k}"] = alpha_pe.reshape(N_CORES, 128, nch_tot * H1)
        oht = np.zeros((N_CORES, 128, nch_tot, 128), np.float16)
        j = np.arange(128)
        for c in range(N_CORES):
            oht[c] = (dst_loc[c][:, :, None] == j[None, None, :])
        Hh[f"ohtab{k}"] = oht.reshape(N_CORES, 128, nch_tot * 128)
        batch = inp[f"batch{k}"]
        bounds = np.searchsorted(batch, np.arange(N_GRAPHS + 1))
        for c in range(N_CORES):
            lb = bounds[GPC * c:GPC * (c + 1) + 1] - NV * c
            assert lb[0] == 0 and lb[-1] == NV, f"graphs not aligned: {lb}"
        meta[f"bounds{k}"] = (bounds[:GPC + 1]).tolist()

    # W1aug: row 78 = b1 + 1 (ones-row trick: PSUM = x@W1 + b1 + 1 directly)
    W1aug = np.zeros((D0 + 1, H1 * C1), f16)
    W1aug[:D0] = W1.astype(f16)
    W1aug[D0] = (inp["b1"].astype(np.float32) + 1.0).astype(f16)
    Hh["W1aug"] = W1aug

    W2 = inp["W2"].astype(np.float32)
    wvs = W2 @ inp["a_src2"][0].astype(np.float32)
    wvd = W2 @ inp["a_dst2"][0].astype(np.float32)
    W2aug = np.zeros((H1 * C1, 516), f16)
    W2aug[:, 0:512] = W2.astype(f16)
    W2aug[:, 512] = wvs.astype(f16)
    W2aug[:, 513] = wvd.astype(f16)
    Hh["W2aug"] = W2aug
    # h1 stored as elu+1 -> h2p' = h2p + colsum(W2); asrc2' = asrc2+sum(wvs)
    meta["c0"] = float(-(wvs.sum() + wvd.sum()))
    b2p1 = inp["b2"].astype(np.float32) - W2.sum(axis=0) + 1.0
    Hh["b2p1"] = b2p1.reshape(512, 1)

    Wg = inp["Wg"].astype(np.float32)
    bg_eff = inp["bg"].astype(np.float32) - Wg.sum(axis=0)
    heads = {"g": ("Wg", bg_eff, 512, 128),
             "r1": ("Wr1", inp["br1"], 954, 2048),
             "r2": ("Wr2", inp["br2"], 2048, 512),
             "r3": ("Wr3", inp["br3"], 512, 256),
             "f1": ("Wf1", inp["bf1"], 512, 1024),
             "f2": ("Wf2", inp["bf2"], 1024, 512),
             "f3": ("Wf3", inp["bf3"], 512, 128),
             "o": ("Wo", inp["bo"], 128, 2)}
    for nm, (wn, bv, KD, MD) in heads.items():
        KD0 = inp[wn].shape[0]
        Kp = int(np.ceil(KD0 / 128)) * 128
        W = np.zeros((Kp, MD), f16)
        W[:KD0] = inp[wn].astype(f16)
        Hh[f"W{nm}"] = W
        Hh[f"b{nm}"] = np.asarray(bv, np.float32).reshape(MD, 1)
    cell = inp["cell"].astype(np.float32)
    cT = np.zeros((N_CORES, 1024, GPC), np.float32)
    for c in range(N_CORES):
        cT[c, :954] = cell[GPC * c:GPC * (c + 1)].T
    Hh["cellT"] = cT
    iota = np.broadcast_to(np.arange(128, dtype=np.float32)[None, :],
                           (128, 128)).copy()
    Hh["iotain"] = iota
    return Hh, meta


# ---------------------------------------------------------------- program
def build(Hh, meta, debug_outputs=False, repeat=1, ab=frozenset()):
    nc = bacc.Bacc("TRN2", target_bir_lowering=False, debug=False,
                   num_devices=N_CORES)

    def din(name, dtype):
        arr = Hh[name]
        return nc.dram_tensor(name, list(arr.shape), dtype,
                              kind="ExternalInput").ap()

    xtab = {k: din(f"xtab{k}", F16) for k in (1, 2)}
    W1aug = din("W1aug", F16)
    W2aug = din("W2aug", F16)
    b2p1 = din("b2p1", F32)
    iotain = din("iotain", F32)
    Wt = {nm: din(f"W{nm}", F16)
          for nm in ("g", "r1", "r2", "r3", "f1", "f2", "f3", "o")}
    Bt = {nm: din(f"b{nm}", F32)
          for nm in ("g", "r1", "r2", "r3", "f1", "f2", "f3", "o")}
    percore = {}
    for k in (1, 2):
        for nm, dtp in (("l1ix", I16), ("l2dx", I16), ("alphat", F16),
                        ("ohtab", F16)):
            arr = Hh[f"{nm}{k}"]
            percore[f"{nm}{k}"] = nc.dram_tensor(
                f"{nm}{k}", list(arr.shape[1:]), dtp, kind="ExternalInput").ap()
    cellT = nc.dram_tensor("cellT", list(Hh["cellT"].shape[1:]), F32,
                           kind="ExternalInput").ap()

    outT = nc.dram_tensor("outT", [2, GPC], F32, kind="ExternalOutput").ap()
    if debug_outputs:
        dbg_h1T = nc.dram_tensor("dbg_h1T", [128, 768], F32,
                                 kind="ExternalOutput").ap()
        dbg_loc = nc.dram_tensor("dbg_loc", [128, T2], F32,
                                 kind="ExternalOutput").ap()
        dbg_o2T = nc.dram_tensor("dbg_o2T", [128, 768], F32,
                                 kind="ExternalOutput").ap()
        dbg_vT = nc.dram_tensor("dbg_vT", [128, GPC * 2], F32,
                                kind="ExternalOutput").ap()

    loc2 = {k: nc.dram_tensor(f"loc2_{k}", [NV, T2], F16).ap() for k in (1, 2)}
    shared2 = {k: nc.dram_tensor(f"shared2_{k}", [N_NODES, T2], F16,
                                 addr_space="Shared").ap() for k in (1, 2)}

    n_ch = {k: meta[f"n_ch{k}"] for k in (1, 2)}
    nch_tot = {k: sum(n_ch[k]) for k in (1, 2)}
    bounds = meta["bounds1"]
    assert meta["bounds2"] == bounds
    c0 = meta["c0"]

    with tile.TileContext(nc) as tc:
        with (
            tc.tile_pool(name="const", bufs=1) as const,
            tc.tile_pool(name="w1pool", bufs=1) as w1pool,
            tc.tile_pool(name="h1pool", bufs=1) as h1pool,
            tc.tile_pool(name="aggtp", bufs=1) as aggtp,
            tc.tile_pool(name="o2pool", bufs=1) as o2pool,
            tc.tile_pool(name="vpool", bufs=1) as vpool,
            tc.tile_pool(name="sb", bufs=2) as sb,
            tc.tile_pool(name="hw", bufs=1) as hw,
        ):
            ident = const.tile([128, 128], F32)
            make_identity(nc, ident)
            identF = const.tile([128, 128], F16)
            nc.vector.tensor_copy(out=identF[:], in_=ident[:])

            negone = const.tile([128, 1], F32)
            nc.vector.memset(negone[:], -1.0)
            b2p1t = const.tile([128, 4], F32)
            nc.sync.dma_start(out=b2p1t[:],
                              in_=b2p1.rearrange("(a p) o -> p (a o)", p=128))

            ixt = {}
            for k in (1, 2):
                NCH = nch_tot[k]
                for nm, dtp in (("l1ix", I16), ("l2dx", I16)):
                    t = const.tile([128, NCH * 8], dtp, tag=f"{nm}{k}")
                    nc.sync.dma_start(out=t[:], in_=percore[f"{nm}{k}"][:])
                    ixt[f"{nm}{k}"] = t
                t = const.tile([128, NCH * H1], F16, tag=f"alphat{k}")
                nc.sync.dma_start(out=t[:], in_=percore[f"alphat{k}"][:])
                ixt[f"alphat{k}"] = t

            h1T = [h1pool.tile([128, 768], F16, tag=f"h1T{i}", name=f"h1T{i}")
                   for i in range(80)]
            o2T = [o2pool.tile([128, 768], F16, tag=f"o2T{i}", name=f"o2T{i}")
                   for i in range(4)]
            vT = {k: vpool.tile([128, GPC], F16, tag=f"vT{k}", name=f"vT{k}")
                  for k in (1, 2)}
            aggT = [aggtp.tile([D0 + 1, 768], F16, tag=f"aggT{h}",
                               name=f"aggT{h}") for h in range(H1)]

            def lrelu_exp(src_ap, n, tag, out_dt=F32):
                t1 = sb.tile([128, n], F32, tag=f"{tag}_t1")
                nc.vector.tensor_scalar_mul(t1[:], src_ap, NEG_SLOPE)
                t2 = sb.tile([128, n], F32, tag=f"{tag}_t2")
                nc.vector.tensor_tensor(out=t2[:], in0=t1[:], in1=src_ap,
                                        op=Alu.max)
                w = sb.tile([128, n], out_dt, tag=f"{tag}_w")
                nc.scalar.activation(w[:], t2[:], Act.Exp)
                return w

            # ---------------- phase A: L1 aggregation (host alpha + oh)
            def phase_A(k):
                alphat = ixt[f"alphat{k}"]
                if "noA" in ab:
                    for h in range(H1):
                        nc.vector.memset(aggT[h][:], 0.01)
                    return
                with (
                    tc.tile_pool(name=f"gx{k}", bufs=2) as gxp,
                    tc.tile_pool(name=f"ohp{k}", bufs=2) as ohp,
                    tc.tile_pool(name=f"psagg{k}", bufs=2, space="PSUM") as psagg,
                    tc.tile_pool(name=f"pstr{k}", bufs=2, space="PSUM") as pstr,
                ):
                    ch0 = 0
                    for b in range(NBLK):
                        nchb = n_ch[k][b]
                        gx = gxp.tile([128, nchb, XTW], F16, tag="gx")
                        if "noGA" in ab:
                            nc.vector.memset(gx[:], 0.01)
                        else: nc.gpsimd.dma_gather(
                            gx[:], xtab[k][:, :],
                            ixt[f"l1ix{k}"][:, ch0 * 8:(ch0 + nchb) * 8],
                            nchb * 128, nchb * 128, XTW)
                        oht = ohp.tile([128, nchb, 128], F16, tag="oht")
                        nc.sync.dma_start(
                            out=oht[:],
                            in_=percore[f"ohtab{k}"][:, ch0 * 128:
                                                     (ch0 + nchb) * 128])
                        ps = psagg.tile([128, H1 * 79], F32, tag="agg")
                        for ci in range(nchb):
                            ch = ch0 + ci
                            xg = gx[:, ci, :]
                            wxg = sb.tile([128, H1, 79], F16, tag="wxg")
                            nc.vector.tensor_tensor(
                                out=wxg[:],
                                in0=xg[:, 0:79].rearrange(
                                    "p (o f) -> p o f", o=1).broadcast_to(
                                        [128, H1, 79]),
                                in1=alphat[:, ch * H1:(ch + 1) * H1].rearrange(
                                    "p (h o) -> p h o", o=1).broadcast_to(
                                        [128, H1, 79]),
                                op=Alu.mult)
                            wxg2 = wxg[:].rearrange("p h f -> p (h f)")
                            nc.tensor.matmul(ps[:, 0:512], oht[:, ci, :],
                                             wxg2[:, 0:512],
                                             start=(ci == 0), stop=False)
                            nc.tensor.matmul(ps[:, 512:790], oht[:, ci, :],
                                             wxg2[:, 512:790],
                                             start=(ci == 0),
                                             stop=(ci == nchb - 1))
                        for h in range(H1):
                            sc = sb.tile([128, D0 + 1], F16, tag="sc")
                            nc.vector.tensor_copy(
                                out=sc[:], in_=ps[:, h * 79:(h + 1) * 79])
                            tp = pstr.tile([D0 + 1, 128], F16, tag="tp")
                            nc.tensor.transpose(out=tp[:], in_=sc[:],
                                                identity=identF[:])
                            nc.scalar.copy(
                                out=aggT[h][:, b * 128:(b + 1) * 128],
                                in_=tp[:])
                        ch0 += nchb

            # ---------------- phase B: L1 finalize (h1 = elu(x@W1+b1)+1)
            def phase_B(k):
                with tc.tile_pool(name=f"psfin{k}", bufs=2,
                                  space="PSUM") as psfin:
                    for h in range(H1):
                        for cc in range(8):
                            i = h * 8 + cc
                            pf = psfin.tile([128, 768], F32, tag="fin")
                            w1c = sb.tile([D0 + 1, 128], F16, tag="w1c")
                            nc.sync.dma_start(
                                out=w1c[:],
                                in_=W1aug[:, i * 128:(i + 1) * 128])
                            lhs = w1c[:]
                            nc.tensor.matmul(pf[:, 0:512], lhs,
                                             aggT[h][:, 0:512],
                                             start=True, stop=False)
                            nc.tensor.matmul(pf[:, 512:768], lhs,
                                             aggT[h][:, 512:768],
                                             start=True, stop=True)
                            if "noBelu" in ab:
                                nc.scalar.copy(out=h1T[i][:], in_=pf[:])
                                continue
                            # pf = x + b1 + 1 ; y = max(pf, exp(min(pf,1)-1))
                            t = sb.tile([128, 768], F16, tag="elu_t")
                            nc.vector.tensor_scalar(
                                out=t[:], in0=pf[:], scalar1=1.0,
                                scalar2=None, op0=Alu.min)
                            e = sb.tile([128, 768], F16, tag="elu_e")
                            nc.scalar.activation(e[:], t[:], Act.Exp,
                                                 bias=negone[:, 0:1])
                            nc.vector.tensor_tensor(out=h1T[i][:], in0=pf[:],
                                                    in1=e[:], op=Alu.max)
                    if debug_outputs and k == 1:
                        dt_ = hw.tile([128, 768], F32, tag="dbgT")
                        nc.vector.tensor_copy(out=dt_[:], in_=h1T[0][:])
                        nc.sync.dma_start(out=dbg_h1T[:], in_=dt_[:])

            # ---------------- phase C: L2 big matmul + table build
            def phase_C(k):
                with tc.tile_pool(name=f"psmm{k}", bufs=1,
                                  space="PSUM") as psmm:
                    pm = [psmm.tile([128, 512], F32, tag=f"pm{m}",
                                    name=f"pm{m}") for m in range(6)]
                    with tc.tile_pool(name=f"psaux{k}", bufs=1,
                                      space="PSUM") as psaux:
                        pax = psaux.tile([2, 768], F32, tag="pax")
                        if "noC" in ab:
                            for m in range(6):
                                nc.vector.memset(pm[m][:], 0.01)
                            nc.vector.memset(pax[:], 0.01)
                        for kc in ([] if "noC" in ab else range(80)):
                            wchunk = sb.tile([128, 516], F16, tag="w2c")
                            nc.sync.dma_start(
                                out=wchunk[:],
                                in_=W2aug[kc * 128:(kc + 1) * 128, :])
                            for m in range(6):
                                lhs = h1T[kc][:, m * 128:(m + 1) * 128]
                                nc.tensor.matmul(pm[m][:], lhs,
                                                 wchunk[:, 0:512],
                                                 start=(kc == 0),
                                                 stop=(kc == 79))
                            nc.tensor.matmul(pax[:, 0:512],
                                             wchunk[:, 512:514],
                                             h1T[kc][:, 0:512],
                                             start=(kc == 0), stop=(kc == 79),
                                             skip_group_check=True)
                            nc.tensor.matmul(pax[:, 512:768],
                                             wchunk[:, 512:514],
                                             h1T[kc][:, 512:768],
                                             start=(kc == 0), stop=(kc == 79),
                                             skip_group_check=True)
                        auxS = sb.tile([2, 768], F32, tag="auxS")
                        nc.scalar.copy(out=auxS[:], in_=pax[:])
                    with tc.tile_pool(name=f"pstrm{k}", bufs=1,
                                      space="PSUM") as pstrm:
                      for m in range(6):
                        nrow = 128 if m < 5 else NV - 640
                        trm = pstrm.tile([128, 2], F32, tag="trm")
                        nc.tensor.transpose(
                            out=trm[:], in_=auxS[:, m * 128:(m + 1) * 128],
                            identity=ident[0:2, 0:2])
                        loc = sb.tile([128, T2], F16, tag="loc")
                        nc.scalar.copy(out=loc[:, 0:512], in_=pm[m][:])
                        nc.vector.tensor_scalar(
                            out=loc[:, 512:514].bitcast(F32),
                            in0=trm[:, 0:1], scalar1=c0,
                            scalar2=None, op0=Alu.add)
                        nc.vector.tensor_copy(
                            out=loc[:, 514:516].bitcast(F32),
                            in_=trm[:, 1:2])
                        nc.sync.dma_start(
                            out=loc2[k][m * 128:m * 128 + nrow, :],
                            in_=loc[0:nrow, :])
                        if debug_outputs and k == 1 and m == 0:
                            dlf = hw.tile([128, 768], F32, tag="dbgT", name="dlf")
                            dl = dlf[:, 0:T2]
                            nc.vector.tensor_copy(out=dl, in_=loc[:])
                            nc.sync.dma_start(out=dbg_loc[:], in_=dl)

            def phase_AG(k):
                if "noAG" in ab:
                    nc.sync.dma_start(out=shared2[k][0:NV, :], in_=loc2[k][:])
                    return
                nc.gpsimd.collective_compute(
                    "AllGather", Alu.bypass,
                    replica_groups=[list(range(N_CORES))],
                    ins=[loc2[k][:].opt()], outs=[shared2[k][:].opt()])

            # ---------------- phase D: L2 attention + aggregation
            def phase_D(k):
                if "noD" in ab:
                    for cc in range(4):
                        nc.vector.memset(o2T[cc][:], 0.01)
                    return
                with (
                    tc.tile_pool(name=f"gh{k}", bufs=2) as ghp,
                    tc.tile_pool(name=f"ga{k}", bufs=2) as gap,
                    tc.tile_pool(name=f"ohd{k}", bufs=2) as ohdp,
                    tc.tile_pool(name=f"psag2{k}", bufs=2, space="PSUM") as psag2,
                    tc.tile_pool(name=f"pstr2{k}", bufs=2, space="PSUM") as pstr2,
                ):
                    ch0 = 0
                    for b in range(NBLK):
                        nchb = n_ch[k][b]
                        ohd = ohdp.tile([128, nchb, 128], F16, tag="ohd")
                        nc.sync.dma_start(
                            out=ohd[:],
                            in_=percore[f"ohtab{k}"][:, ch0 * 128:
                                                     (ch0 + nchb) * 128])
                        pieces = []  # (p0, np_, gh, ga)
                        for p0 in range(0, nchb, 4):
                            np_ = min(4, nchb - p0)
                            c0_ = ch0 + p0
                            gh = ghp.tile([128, np_, T2], F16, tag="gh")
                            if "noGA" in ab:
                                nc.vector.memset(gh[:], 0.01)
                            elif True: nc.gpsimd.dma_gather(
                                gh[:], shared2[k][:, :],
                                ixt[f"l1ix{k}"][:, c0_ * 8:(c0_ + np_) * 8],
                                np_ * 128, np_ * 128, T2)
                            ga = gap.tile([128, np_, 128], F16, tag="ga")
                            if "noGA" in ab:
                                nc.vector.memset(ga[:], 0.01)
                            elif True: nc.gpsimd.dma_gather(
                                ga[:], shared2[k][:, 512:T2],
                                ixt[f"l2dx{k}"][:, c0_ * 8:(c0_ + np_) * 8],
                                np_ * 128, np_ * 128, 128, elem_step=T2)
                            pieces.append((p0, np_, gh, ga))
                        ps5 = psag2.tile([128, 512], F32, tag="agg2")
                        psd = psag2.tile([128, 8], F32, tag="agg2d")
                        # batched attention weights per piece (4 chunks/op)
                        w2hs = []
                        for (p0, np_, gh, ga) in pieces:
                            e0 = sb.tile([128, 4], F32, tag="e2b")
                            nc.vector.tensor_tensor(
                                out=e0[:, 0:np_],
                                in0=gh[:, :, 512:514].bitcast(F32).rearrange(
                                    "p n o -> p (n o)"),
                                in1=ga[:, :, 2:4].bitcast(F32).rearrange(
                                    "p n o -> p (n o)"), op=Alu.add)
                            w2 = lrelu_exp(e0[:, 0:np_], np_, "l2")
                            w2h = sb.tile([128, 4], F16, tag="w2h")
                            nc.vector.tensor_copy(out=w2h[:, 0:np_], in_=w2[:])
                            w2hs.append((w2, w2h))
                        for ci in range(nchb):
                            pi = ci // 4
                            p0, np_, gh, ga = pieces[pi]
                            w2, w2h = w2hs[pi]
                            cl = ci - p0
                            wh = sb.tile([128, 512], F16, tag="wh")
                            nc.vector.tensor_scalar(
                                out=wh[:], in0=gh[:, cl, 0:512],
                                scalar1=w2[:, cl:cl + 1], scalar2=None,
                                op0=Alu.mult)
                            nc.tensor.matmul(ps5[:], ohd[:, ci, :],
                                             wh[:],
                                             start=(ci == 0),
                                             stop=(ci == nchb - 1))
                            nc.tensor.matmul(psd[:, 0:1], ohd[:, ci, :],
                                             w2h[:, cl:cl + 1],
                                             start=(ci == 0),
                                             stop=(ci == nchb - 1),
                                             skip_group_check=True)
                        den = sb.tile([128, 1], F32, tag="dn2")
                        nc.vector.tensor_scalar_max(den[:], psd[:, 0:1], 1e-30)
                        rec = sb.tile([128, 1], F32, tag="rc2")
                        nc.vector.reciprocal(out=rec[:], in_=den[:])
                        for cc in range(4):
                            sc = sb.tile([128, 128], F16, tag="sc2")
                            nc.vector.tensor_scalar(
                                out=sc[:], in0=ps5[:, cc * 128:(cc + 1) * 128],
                                scalar1=rec[:, 0:1], scalar2=None,
                                op0=Alu.mult)
                            tp = pstr2.tile([128, 128], F16, tag="tp2")
                            nc.tensor.transpose(out=tp[:], in_=sc[:],
                                                identity=identF[:])
                            # y = max(tp+b2p1, exp(min(tp+b2p1,1)-1))
                            x1 = sb.tile([128, 128], F16, tag="x1")
                            nc.vector.tensor_scalar(
                                out=x1[:], in0=tp[:],
                                scalar1=b2p1t[:, cc:cc + 1], scalar2=None,
                                op0=Alu.add)
                            t2 = sb.tile([128, 128], F16, tag="t2")
                            nc.vector.tensor_scalar(
                                out=t2[:], in0=tp[:],
                                scalar1=b2p1t[:, cc:cc + 1], scalar2=1.0,
                                op0=Alu.add, op1=Alu.min)
                            e2 = sb.tile([128, 128], F16, tag="e2e")
                            nc.scalar.activation(e2[:], t2[:], Act.Exp,
                                                 bias=negone[:, 0:1])
                            nc.vector.tensor_tensor(
                                out=o2T[cc][:, b * 128:(b + 1) * 128],
                                in0=x1[:], in1=e2[:], op=Alu.max)
                        ch0 += nchb
                    if debug_outputs and k == 1:
                        do_ = hw.tile([128, 768], F32, tag="dbgT")
                        nc.vector.tensor_copy(out=do_[:], in_=o2T[0][:])
                        nc.sync.dma_start(out=dbg_o2T[:], in_=do_[:])

            # ---------------- phase E: global max pool + Wg
            def phase_E(k):
                with tc.tile_pool(name=f"psg{k}", bufs=2, space="PSUM") as psg:
                    gT = hw.tile([128, 4, GPC], F16, tag=f"gT{k}")
                    for cc in range(4):
                        for g in range(GPC):
                            nc.vector.tensor_reduce(
                                out=gT[:, cc, g:g + 1],
                                in_=o2T[cc][:, bounds[g]:bounds[g + 1]],
                                axis=mybir.AxisListType.X, op=Alu.max)
                    pg = psg.tile([128, GPC], F32, tag="pg")
                    for kc in range(4):
                        wgt = sb.tile([128, 128], F16, tag="dw")
                        nc.sync.dma_start(
                            out=wgt[:], in_=Wt["g"][kc * 128:(kc + 1) * 128, :])
                        nc.tensor.matmul(pg[:], wgt[:], gT[:, kc, :],
                                         start=(kc == 0), stop=(kc == 3))
                    bgt = sb.tile([128, 1], F32, tag="bcol")
                    nc.sync.dma_start(out=bgt[:], in_=Bt["g"][:])
                    nc.scalar.activation(vT[k][:], pg[:], Act.Relu,
                                         bias=bgt[:, 0:1])

            # ---------------- head helpers
            def l2norm_scale(xtiles, tag):
                n = len(xtiles)
                with tc.tile_pool(name=f"psn{tag}", bufs=1, space="PSUM") as psn:
                    pn = psn.tile([1, GPC], F32, tag=f"pn{tag}")
                    ones = const.tile([128, 1], F16, tag=f"one{tag}")
                    nc.vector.memset(ones[:], 1.0)
                    for i in range(n):
                        sq = sb.tile([128, GPC], F16, tag=f"sq{tag}")
                        nc.scalar.activation(sq[:], xtiles[i][:], Act.Square)
                        nc.tensor.matmul(pn[:], ones[:], sq[:],
                                         start=(i == 0), stop=(i == n - 1))
                    nrm = sb.tile([1, GPC], F32, tag=f"nr{tag}")
                    nc.scalar.activation(nrm[:], pn[:], Act.Sqrt)
                    nc.vector.tensor_scalar_max(nrm[:], nrm[:], 1e-12)
                    rcp = sb.tile([1, GPC], F32, tag=f"rcn{tag}")
                    nc.vector.reciprocal(out=rcp[:], in_=nrm[:])
                    rb = hw.tile([128, GPC], F32, tag=f"rb{tag}")
                    nc.gpsimd.partition_broadcast(rb[:], rcp[:])
                    outs = []
                    for i in range(n):
                        o = hw.tile([128, GPC], F16, tag=f"no{tag}{i}")
                        nc.vector.tensor_tensor(out=o[:], in0=xtiles[i][:],
                                                in1=rb[:], op=Alu.mult)
                        outs.append(o)
                    return outs

            def dense(xtiles, nm, md, act=True, out_f32=False):
                kc = len(xtiles)
                mc = (md + 127) // 128
                outs = []
                bt = sb.tile([min(128, md), (md + 127) // 128], F32,
                             tag=f"bt{nm}")
                nc.sync.dma_start(
                    out=bt[:],
                    in_=Bt[nm].rearrange("(a p) o -> p (a o)",
                                         p=min(128, md)))
                with tc.tile_pool(name=f"psd{nm}", bufs=2, space="PSUM") as psd:
                    for m in range(mc):
                        mw = min(128, md - m * 128)
                        pd = psd.tile([mw, GPC], F32, tag=f"pd{nm}")
                        for i in range(kc):
                            dw = sb.tile([128, mw], F16, tag="dw")
                            nc.sync.dma_start(
                                out=dw[:],
                                in_=Wt[nm][i * 128:(i + 1) * 128,
                                           m * 128:m * 128 + mw])
                            nc.tensor.matmul(pd[:], dw[:], xtiles[i][:],
                                             start=(i == 0), stop=(i == kc - 1))
                        o = hw.tile([mw, GPC], F32 if out_f32 else F16,
                                    tag=f"do{nm}{m}")
                        if act:
                            nc.scalar.activation(o[:], pd[:], Act.Relu,
                                                 bias=bt[0:mw, m:m + 1])
                        else:
                            nc.vector.tensor_scalar(out=o[:], in0=pd[:],
                                                    scalar1=bt[0:mw, m:m + 1],
                                                    scalar2=None, op0=Alu.add)
                        outs.append(o)
                return outs

            def _body():
                phase_A(1)
                phase_B(1)
                phase_C(1)
                phase_AG(1)
                phase_A(2)
                phase_B(2)
                phase_C(2)
                phase_D(1)
                phase_E(1)
                phase_AG(2)
                # cell MLP overlaps AG(2)
                cT_t = []
                for i in range(8):
                    t = hw.tile([128, GPC], F32, tag=f"cT{i}")
                    nc.sync.dma_start(out=t[:],
                                      in_=cellT[i * 128:(i + 1) * 128, :])
                    cT_t.append(t)
                cn = l2norm_scale(cT_t, "c")
                r1 = dense(cn, "r1", 2048)
                r2 = dense(r1, "r2", 512)
                r3 = dense(r2, "r3", 256)
                phase_D(2)
                phase_E(2)
                if "nohead" in ab:
                    zo = hw.tile([2, GPC], F32, tag="zo")
                    nc.vector.tensor_copy(out=zo[:], in_=vT[1][0:2, :])
                    nc.sync.dma_start(out=outT[:], in_=zo[:])
                    return
                if debug_outputs:
                    vt1 = hw.tile([128, GPC], F32, tag="vdbg1")
                    nc.vector.tensor_copy(out=vt1[:], in_=vT[1][:])
                    nc.sync.dma_start(out=dbg_vT[:, 0:GPC], in_=vt1[:])
                    vt2 = hw.tile([128, GPC], F32, tag="vdbg2")
                    nc.vector.tensor_copy(out=vt2[:], in_=vT[2][:])
                    nc.sync.dma_start(out=dbg_vT[:, GPC:2 * GPC], in_=vt2[:])
                xc_t = []
                for j, src_t in enumerate((vT[1], vT[2], r3[0], r3[1])):
                    t = hw.tile([128, GPC], F32, tag=f"xc{j}")
                    nc.vector.tensor_copy(out=t[:], in_=src_t[:])
                    xc_t.append(t)
                xn = l2norm_scale(xc_t, "x")
                f1 = dense(xn, "f1", 1024)
                f2 = dense(f1, "f2", 512)
                f3 = dense(f2, "f3", 128)
                fo = dense(f3, "o", 2, act=False, out_f32=True)
                nc.sync.dma_start(out=outT[:], in_=fo[0][:])

            for _rep in range(repeat):
                _body()

    nc.compile()
    return nc


def make_in_maps(Hh):
    ins = []
    percore_keys = ("l1ix1", "l2dx1", "alphat1", "ohtab1",
                    "l1ix2", "l2dx2", "alphat2", "ohtab2", "cellT")
    for c in range(N_CORES):
        m = {}
        for k, v in Hh.items():
            if k in percore_keys:
                m[k] = np.ascontiguousarray(v[c])
            else:
                m[k] = v
        ins.append(m)
    return ins


# ---------------------------------------------------------------------- runner
import time
import jax
from jax.sharding import Mesh, PartitionSpec
from jax.experimental.shard_map import shard_map
from concourse import bass2jax
from concourse.bass2jax import _bass_exec_p, install_neuronx_cc_hook


class SpmdRunner:
    def __init__(self, nc, n_cores: int):
        install_neuronx_cc_hook()
        self.nc = nc
        self.n_cores = n_cores
        partition_name = nc.partition_id_tensor.name if nc.partition_id_tensor else None
        in_names, out_names, out_avals, zero_outs = [], [], [], []
        for alloc in nc.m.functions[0].allocations:
            if not isinstance(alloc, mybir.MemoryLocationSet):
                continue
            name = alloc.memorylocations[0].name
            if alloc.kind == "ExternalInput":
                if name != partition_name:
                    in_names.append(name)
            elif alloc.kind == "ExternalOutput":
                out_names.append(name)
                shape = tuple(alloc.tensor_shape)
                dtype = mybir.dt.np(alloc.dtype)
                out_avals.append(jax.core.ShapedArray(shape, dtype))
                zero_outs.append(np.zeros(shape, dtype))
        self.in_names = list(in_names)
        self.out_names = out_names
        self.out_avals = out_avals
        self.zero_outs = zero_outs
        n_params = len(in_names)
        self.n_params = n_params
        all_in_names = list(in_names) + list(out_names)
        if partition_name is not None:
            all_in_names.append(partition_name)

        def _body(*args):
            operands = list(args)
            if partition_name is not None:
                operands.append(bass2jax.partition_id_tensor())
            outs = _bass_exec_p.bind(
                *operands,
                out_avals=tuple(out_avals),
                in_names=tuple(all_in_names),
                out_names=tuple(out_names),
                lowering_input_output_aliases=(),
                sim_require_finite=True,
                sim_require_nnan=True,
                nc=nc,
            )
            return tuple(outs)

        donate = tuple(range(n_params, n_params + len(out_names)))
        devices = jax.devices()[:n_cores]
        mesh = Mesh(np.asarray(devices), ("core",))
        in_specs = (PartitionSpec("core"),) * (n_params + len(out_names))
        out_specs = (PartitionSpec("core"),) * len(out_names)
        self._fn = jax.jit(
            shard_map(_body, mesh=mesh, in_specs=in_specs, out_specs=out_specs,
                      check_rep=False),
            donate_argnums=donate, keep_unused=True)

    def _concat_inputs(self, in_maps):
        per_core = [[np.asarray(m[n]) for n in self.in_names] for m in in_maps]
        return [np.concatenate([per_core[c][i] for c in range(self.n_cores)],
                               axis=0)
                for i in range(self.n_params)]

    def _zeros(self):
        return [np.zeros((self.n_cores * z.shape[0], *z.shape[1:]), z.dtype)
                for z in self.zero_outs]

    def run(self, in_maps):
        concat_in = self._concat_inputs(in_maps)
        outs = self._fn(*concat_in, *self._zeros())
        res = []
        for c in range(self.n_cores):
            d = {}
            for i, name in enumerate(self.out_names):
                d[name] = np.asarray(outs[i]).reshape(
                    self.n_cores, *self.out_avals[i].shape)[c]
            res.append(d)
        return res

    def time(self, in_maps, iters=20, warmup=3, inner=5):
        concat_in = [jax.device_put(x) for x in self._concat_inputs(in_maps)]
        times = []
        for it in range(warmup + iters):
            zs = [self._zeros() for _ in range(inner)]
            t0 = time.perf_counter()
            outs = None
            for k in range(inner):
                outs = self._fn(*concat_in, *zs[k])
            jax.block_until_ready(outs)
            dt = (time.perf_counter() - t0) / inner
            if it >= warmup:
                times.append(dt)
        return min(times), times


# ---------------------------------------------------------------- entry point
_CACHE = {}


def _get_runner(Hh, meta):
    key = (tuple(meta["n_ch1"]), tuple(meta["n_ch2"]), tuple(meta["bounds1"]))
    ent = _CACHE.get(key)
    if ent is None:
        nc = build(Hh, meta)
        ent = SpmdRunner(nc, N_CORES)
        _CACHE[key] = ent
    return ent


def kernel(**inputs):
    Hh, meta = host_prep(inputs)
    runner = _get_runner(Hh, meta)
    res = runner.run(make_in_maps(Hh))
    out = np.concatenate([res[c]["outT"].T for c in range(N_CORES)], axis=0)
    return out.astype(np.float32)
